# revision 31
# baseline (speedup 1.0000x reference)
"""Trainium2 Bass kernel for nn_ABCFramework_17755394802208.

Conv backbone (6x 3x3 SAME convs + 2 maxpools, 256^2 -> 64^2) feeding full
self-attention over N=4096 tokens with a Swin-style relative-position bias.

Sharding: 8 cores = (batch b in {0,1}) x (head h in {0..3}); each core runs the
conv backbone for its batch, projects q/k/v for its head, and computes full
attention for its (b, h). Output slices are gathered/reassembled on host.

The relative-position bias uses the block-Toeplitz structure of the Swin bias:
bias[n, m] = C[xn-xm+63, yn-ym+63] with C a 127x127 slice of the bias table.
Each SBUF partition p (key m within a 128-chunk) holds a contiguous 65KB
window W[p, t] = cfbuf[off_p + t] of the compact table, so every bias tile of
S^T is a plain strided slice of W; the add happens on the TensorEngine by
accumulating identity @ W_slice into the logits PSUM.
"""
import sys

sys.path.insert(0, '/opt/trn_rl_repo')

import numpy as np

NUM_HEADS = 4
DIM_HEAD = 64
TABLE_M = 160
B = 2
N = 4096          # tokens (64 x 64)
GRID = 64
NCH = 32          # m-chunks of 128 keys
NQC = 8           # n-chunks of 512 queries
CF = 8192         # 127 * 64 compact-table window length
CFBUF = 16576     # padded DRAM buffer (191 front pad + CF + tail pad)
SUPER = 2048      # im2col super-chunk (4 matmul chunks)

# conv layer configs: (Cin, Cout, H, W)
CONVS = [(1, 3, 256, 256), (3, 3, 256, 256),
         (3, 6, 128, 128), (6, 6, 128, 128),
         (6, 9, 64, 64), (9, 9, 64, 64)]


def _lay(Hdim, Wdim):
    Wp = Wdim + 2
    G = Wp + 1
    return Wp, G, Hdim * Wp, (Hdim + 2) * Wp + 2 * G  # Wp, guard, npix, buflen


_BUILD_CACHE = {}


def _build(trace_unused=False):
    if 'nc' in _BUILD_CACHE:
        return _BUILD_CACHE['nc']
    import concourse.bass as bass
    import concourse.mybir as mybir
    import concourse.tile as tile
    from concourse import bacc
    from concourse.masks import make_identity

    F32 = mybir.dt.float32
    F32R = mybir.dt.float32r
    AF = mybir.ActivationFunctionType

    import os
    DBG = os.environ.get("KDBG", "") == "1"
    nc = bacc.Bacc("TRN2", target_bir_lowering=False, debug=False, num_devices=8)

    # ---- external inputs (per-core shards prepared on host) ----
    _, _, _, BUF1 = _lay(256, 256)
    x_d = nc.dram_tensor("x", [1, BUF1], F32R, kind="ExternalInput")
    w_d, b_d = [], []
    for i, (ci, co, _, _) in enumerate(CONVS):
        w_d.append(nc.dram_tensor(f"w{i}", [ci * 9, co], F32R, kind="ExternalInput"))
        b_d.append(nc.dram_tensor(f"b{i}", [co], F32, kind="ExternalInput"))
    wq_d = nc.dram_tensor("wq", [9, 64], F32R, kind="ExternalInput")
    wk_d = nc.dram_tensor("wk", [9, 64], F32R, kind="ExternalInput")
    wv_d = nc.dram_tensor("wv", [9, 64], F32R, kind="ExternalInput")
    wa_d = nc.dram_tensor("watlas", [128, CF], F32R, kind="ExternalInput")
    out_d = nc.dram_tensor("out", [64, N], F32, kind="ExternalOutput")
    if DBG:
        dtok_d = nc.dram_tensor("dtok", [9, N], F32, kind="ExternalOutput")
        dq_d = nc.dram_tensor("dq", [64, N], F32, kind="ExternalOutput")
        dk_d = nc.dram_tensor("dk", [64, N], F32, kind="ExternalOutput")
        dv_d = nc.dram_tensor("dv", [128, NCH * 65], F32, kind="ExternalOutput")
        dm_d = []
        for i, (ci, co, Hd, Wd) in enumerate(CONVS):
            _, _, _, bl = _lay(Hd, Wd)
            dm_d.append(nc.dram_tensor(f"dm{i}", [co, bl], F32R, kind="ExternalOutput"))
        _, _, _, _bl3 = _lay(128, 128)
        dp2_d = nc.dram_tensor("dp2", [3, _bl3], F32R, kind="ExternalOutput")
        _, _, _, _bl5 = _lay(64, 64)
        dp4_d = nc.dram_tensor("dp4", [6, _bl5], F32R, kind="ExternalOutput")

    with tile.TileContext(nc) as tc:
        with tc.tile_pool(name="const", bufs=1) as const, \
             tc.tile_pool(name="work", bufs=(2 if DBG else 3)) as work, \
             tc.tile_pool(name="dram", bufs=1, space="DRAM") as dram:

            # ---------------- constants ----------------
            wt, bt = [], []
            for i, (ci, co, _, _) in enumerate(CONVS):
                w_t = const.tile([ci * 9, co], F32R, tag=f"w{i}")
                nc.sync.dma_start(out=w_t, in_=w_d[i][:, :])
                wt.append(w_t)
                b_t = const.tile([co, 1], F32, tag=f"b{i}")
                nc.sync.dma_start(out=b_t, in_=b_d[i][:, None])
                bt.append(b_t)
            wq_t = const.tile([9, 64], F32R, tag="wq")
            wk_t = const.tile([9, 64], F32R, tag="wk")
            wv_t = const.tile([9, 64], F32R, tag="wv")
            nc.sync.dma_start(out=wq_t, in_=wq_d[:, :])
            nc.sync.dma_start(out=wk_t, in_=wk_d[:, :])
            nc.sync.dma_start(out=wv_t, in_=wv_d[:, :])

            ident_f = const.tile([128, 128], F32, tag="idf")
            make_identity(nc, ident_f)
            ident = const.tile([128, 128], F32R, tag="id")
            nc.scalar.copy(out=ident, in_=ident_f)

            ones_f = const.tile([128, 1], F32, tag="onf")
            nc.vector.memset(ones_f, 1.0)
            ones_r = const.tile([1, 64], F32R, tag="onr")
            onesf64 = const.tile([1, 64], F32, tag="onf64")
            nc.vector.memset(onesf64, 1.0)
            nc.scalar.copy(out=ones_r, in_=onesf64)

            zeros = const.tile([9, 768], F32R, tag="zeros")
            nc.vector.memset(zeros.bitcast(F32), 0.0)

            # ---- bias atlas W (host-prepared sliding windows) ----
            W = const.tile([128, CF], F32R, tag="W")
            nc.sync.dma_start(out=W, in_=wa_d[:, :])

            # ---------------- conv backbone ----------------
            maps = []
            for i, (ci, co, Hd, Wd) in enumerate(CONVS):
                _, _, _, bl = _lay(Hd, Wd)
                maps.append(dram.tile([co, bl], F32R, tag=f"m{i}", name=f"m{i}"))
            _, _, _, bl3 = _lay(128, 128)
            p2 = dram.tile([3, bl3], F32R, tag="p2")
            _, _, _, bl5 = _lay(64, 64)
            p4 = dram.tile([6, bl5], F32R, tag="p4")

            relu_par = [0]

            def conv_layer(inten, Cin, Cout, Hd, Wd, w_t, b_t, outten, li):
                Wp, G, NPIX, bl_in = _lay(Hd, Wd)
                _, _, _, bl_out = _lay(Hd, Wd)
                nsup = (NPIX + SUPER - 1) // SUPER
                for si in range(nsup):
                    q0 = si * SUPER
                    sl = min(SUPER, NPIX - q0)
                    col = work.tile([Cin * 9, SUPER], F32R, tag="col")
                    for c in range(Cin):
                        base = c * bl_in + G + q0 - 1
                        src = bass.AP(tensor=inten, offset=base,
                                      ap=[[Wp, 3], [1, 3], [1, sl]])
                        nc.sync.dma_start(out=col[c * 9:(c + 1) * 9, 0:sl], in_=src)
                    for hb in range(0, sl, 2048):
                        hl = min(2048, sl - hb)
                        ro = work.tile([Cout, 2048], F32R, tag="ro", bufs=2)
                        nsub = (hl + 511) // 512
                        for k in range(nsub):
                            o0 = k * 512
                            ol = min(512, hl - o0)
                            pst = psc.tile([Cout, 512], F32, tag="cps")
                            nc.tensor.matmul(pst[:, 0:ol], w_t,
                                             col[:, hb + o0:hb + o0 + ol],
                                             start=True, stop=True)
                            relu_par[0] ^= 1
                            if relu_par[0]:
                                nc.scalar.activation(out=ro[:, o0:o0 + ol],
                                                     in_=pst[:, 0:ol], func=AF.Relu,
                                                     bias=b_t, scale=1.0)
                            else:
                                nc.vector.scalar_tensor_tensor(
                                    out=ro[:, o0:o0 + ol], in0=pst[:, 0:ol],
                                    scalar=b_t, in1=zeros[0:Cout, 0:ol],
                                    op0=mybir.AluOpType.add, op1=mybir.AluOpType.max)
                        dst = bass.AP(tensor=outten, offset=G + Wp + q0 + hb,
                                      ap=[[bl_out, Cout], [1, hl]])
                        nc.gpsimd.dma_start(out=dst, in_=ro[0:Cout, 0:hl])
                # guard zeroing of outten
                gl = G + Wp
                dst = bass.AP(tensor=outten, offset=0, ap=[[bl_out, Cout], [1, gl]])
                nc.sync.dma_start(out=dst, in_=zeros[0:Cout, 0:gl])
                dst = bass.AP(tensor=outten, offset=G + Wp * (Hd + 1),
                              ap=[[bl_out, Cout], [1, gl]])
                nc.sync.dma_start(out=dst, in_=zeros[0:Cout, 0:gl])
                for gc in (0, Wp - 1):
                    dst = bass.AP(tensor=outten, offset=G + Wp + gc,
                                  ap=[[bl_out, Cout], [Wp, Hd]])
                    nc.sync.dma_start(out=dst, in_=zeros[0:Cout, 0:Hd])

            def pool_layer(inten, C, Hd, Wd, outten):
                Wp, G, _, bl_in = _lay(Hd, Wd)
                H2, W2 = Hd // 2, Wd // 2
                Wp2, G2, _, bl_out = _lay(H2, W2)
                P2 = 128 // C  # row-pairs per chunk (per channel)
                for r0 in range(0, H2, P2):
                    rp = min(P2, H2 - r0)
                    t3 = work.tile([C * P2, 2, Wd], F32R, tag="plin")
                    for c in range(C):
                        src = bass.AP(tensor=inten,
                                      offset=c * bl_in + G + (2 * r0 + 1) * Wp + 1,
                                      ap=[[2 * Wp, rp], [Wp, 2], [1, Wd]])
                        nc.sync.dma_start(out=t3[c * P2:c * P2 + rp, :, :], in_=src)
                    m1 = work.tile([C * P2, 2, W2], F32R, tag="plw")
                    nc.vector.tensor_max(m1[:, :, :], t3[:, :, 0::2], t3[:, :, 1::2])
                    m2 = work.tile([C * P2, W2], F32R, tag="plh")
                    nc.vector.tensor_max(m2[:, :], m1[:, 0, :], m1[:, 1, :])
                    for c in range(C):
                        dst = bass.AP(tensor=outten,
                                      offset=c * bl_out + G2 + (r0 + 1) * Wp2 + 1,
                                      ap=[[Wp2, rp], [1, W2]])
                        nc.gpsimd.dma_start(out=dst, in_=m2[c * P2:c * P2 + rp, :])

            # pool outputs never write their guard cells: zero them up front,
            # off the conv critical path
            def zero_guards(outten, C, H2, W2):
                Wp2, G2, _, bl_out = _lay(H2, W2)
                gl = G2 + Wp2
                dst = bass.AP(tensor=outten, offset=0, ap=[[bl_out, C], [1, gl]])
                nc.gpsimd.dma_start(out=dst, in_=zeros[0:C, 0:gl])
                dst = bass.AP(tensor=outten, offset=G2 + Wp2 * (H2 + 1),
                              ap=[[bl_out, C], [1, gl]])
                nc.gpsimd.dma_start(out=dst, in_=zeros[0:C, 0:gl])
                for gc in (0, Wp2 - 1):
                    dst = bass.AP(tensor=outten, offset=G2 + Wp2 + gc,
                                  ap=[[bl_out, C], [Wp2, H2]])
                    nc.gpsimd.dma_start(out=dst, in_=zeros[0:C, 0:H2])

            zero_guards(p2.tensor, 3, 128, 128)
            zero_guards(p4.tensor, 6, 64, 64)

            scope_conv = nc.named_scope("conv"); scope_conv.__enter__()
            with tc.tile_pool(name="psc", bufs=4, space="PSUM") as psc:
                conv_layer(x_d, 1, 3, 256, 256, wt[0], bt[0], maps[0].tensor, 0)
                conv_layer(maps[0].tensor, 3, 3, 256, 256, wt[1], bt[1], maps[1].tensor, 1)
                pool_layer(maps[1].tensor, 3, 256, 256, p2.tensor)
                conv_layer(p2.tensor, 3, 6, 128, 128, wt[2], bt[2], maps[2].tensor, 2)
                conv_layer(maps[2].tensor, 6, 6, 128, 128, wt[3], bt[3], maps[3].tensor, 3)
                pool_layer(maps[3].tensor, 6, 128, 128, p4.tensor)
                conv_layer(p4.tensor, 6, 9, 64, 64, wt[4], bt[4], maps[4].tensor, 4)
                conv_layer(maps[4].tensor, 9, 9, 64, 64, wt[5], bt[5], maps[5].tensor, 5)

            scope_conv.__exit__(None, None, None)
            scope_qkv = nc.named_scope("qkv"); scope_qkv.__enter__()
            # ---------------- tokens + q/k/v ----------------
            tokT = const.tile([9, N], F32R, tag="tok")
            Wp5, G5, _, bl5_ = _lay(64, 64)
            src = bass.AP(tensor=maps[5].tensor, offset=G5 + Wp5 + 1,
                          ap=[[bl5_, 9], [Wp5, 64], [1, 64]])
            nc.sync.dma_start(out=tokT.rearrange("c (h w) -> c h w", w=64), in_=src)

            qT = const.tile([64, N], F32R, tag="qT")
            kT = const.tile([64, N], F32R, tag="kT")
            v_sb = const.tile([128, NCH, 65], F32R, tag="v")

            with tc.tile_pool(name="psq", bufs=2, space="PSUM") as psq:
                for j in range(NQC):
                    ps_q = psq.tile([64, 512], F32, tag="qps")
                    nc.tensor.matmul(ps_q, wq_t, tokT[:, j * 512:(j + 1) * 512],
                                     start=True, stop=True)
                    nc.scalar.activation(out=qT[:, j * 512:(j + 1) * 512], in_=ps_q,
                                         func=AF.Copy, scale=float(DIM_HEAD) ** -0.5)
                    ps_k = psq.tile([64, 512], F32, tag="kps")
                    nc.tensor.matmul(ps_k, wk_t, tokT[:, j * 512:(j + 1) * 512],
                                     start=True, stop=True)
                    nc.scalar.copy(out=kT[:, j * 512:(j + 1) * 512], in_=ps_k)
                for c in range(NCH):
                    ps_v = psq.tile([128, 64], F32, tag="vps")
                    nc.tensor.matmul(ps_v, tokT[:, c * 128:(c + 1) * 128], wv_t,
                                     start=True, stop=True)
                    nc.scalar.copy(out=v_sb[:, c, 0:64], in_=ps_v)
                    nc.vector.tensor_copy(v_sb[:, c, 64:65], ones_f)

            if DBG:
                for i in range(6):
                    nc.sync.dma_start(out=dm_d[i][:, :], in_=maps[i][:, :])
                nc.sync.dma_start(out=dp2_d[:, :], in_=p2[:, :])
                nc.sync.dma_start(out=dp4_d[:, :], in_=p4[:, :])
                dt_ = const.tile([9, N], F32, tag="dbg")
                nc.vector.tensor_copy(dt_, tokT)
                nc.sync.dma_start(out=dtok_d[:, :], in_=dt_)
                dq_ = const.tile([64, N], F32, tag="dbg")
                nc.vector.tensor_copy(dq_, qT)
                nc.sync.dma_start(out=dq_d[:, :], in_=dq_)
                dk_ = const.tile([64, N], F32, tag="dbg")
                nc.vector.tensor_copy(dk_, kT)
                nc.sync.dma_start(out=dk_d[:, :], in_=dk_)
                dv_ = const.tile([128, NCH * 65], F32, tag="dbg")
                nc.vector.tensor_copy(dv_, v_sb.rearrange("p c d -> p (c d)"))
                nc.sync.dma_start(out=dv_d[:, :], in_=dv_)

            scope_qkv.__exit__(None, None, None)
            # ---------------- attention ----------------
            Wv = W
            scope_attn = nc.named_scope("attn"); scope_attn.__enter__()
            with tc.tile_pool(name="pss", bufs=4, space="PSUM") as pss, \
                 tc.tile_pool(name="psa", bufs=2, space="PSUM") as psa, \
                 tc.tile_pool(name="psm", bufs=2, space="PSUM") as psm:
                for j in range(NQC):
                    acc = psa.tile([65, 512], F32, tag="acc")
                    for c in range(NCH):
                        s_ps = pss.tile([128, 512], F32, tag="s")
                        nc.tensor.matmul(s_ps, kT[:, c * 128:(c + 1) * 128],
                                         qT[:, j * 512:(j + 1) * 512],
                                         start=True, stop=True)
                        s0 = (8 * j - 2 * c + 63) * 64
                        lg = work.tile([128, 512], F32, tag="lg", bufs=2)
                        nc.vector.tensor_add(lg, s_ps, Wv[:, s0:s0 + 512])
                        at = work.tile([128, 512], F32R, tag="at", bufs=4)
                        nc.scalar.activation(out=at, in_=lg, func=AF.Exp)
                        nc.tensor.matmul(acc, v_sb[:, c, :], at,
                                         start=(c == 0), stop=(c == NCH - 1))
                    # epilogue: divide by the attention sums (row 64 of acc)
                    sums = work.tile([1, 512], F32, tag="sums", bufs=2)
                    nc.scalar.copy(out=sums, in_=acc[64:65, :])
                    rcp_f = work.tile([1, 512], F32, tag="rcpf", bufs=2)
                    nc.vector.reciprocal_approx_fast(out=rcp_f, in_=sums)
                    rcp = work.tile([1, 512], F32R, tag="rcp", bufs=2)
                    nc.scalar.copy(out=rcp, in_=rcp_f)
                    bc_ps = psm.tile([64, 512], F32, tag="bc")
                    nc.tensor.matmul(bc_ps, ones_r, rcp, start=True, stop=True)
                    bc_sb = work.tile([64, 512], F32, tag="bcs", bufs=2)
                    nc.scalar.copy(out=bc_sb, in_=bc_ps)
                    res = work.tile([64, 512], F32, tag="res", bufs=2)
                    nc.vector.tensor_mul(res, acc[0:64, :], bc_sb)
                    nc.sync.dma_start(out=out_d[:, j * 512:(j + 1) * 512], in_=res)
            scope_attn.__exit__(None, None, None)

    nc.finalize()
    _BUILD_CACHE['nc'] = nc
    return nc


def _prep_inputs(inputs):
    """Build the 8 per-core input maps (pure slicing/layout, no math)."""
    x = np.asarray(inputs['x'], dtype=np.float32)
    qkv_w = np.asarray(inputs['qkv_w'], dtype=np.float32)
    table = np.asarray(inputs['bias_table'], dtype=np.float32)

    _, _, _, BUF1 = _lay(256, 256)
    xbufs = []
    for b in range(B):
        pad = np.zeros((258, 258), np.float32)
        pad[1:257, 1:257] = x[b, 0]
        buf = np.zeros((1, BUF1), np.float32)
        g1 = 258 + 1 + 2 * 0  # G = Wp+1 = 259
        G = 259
        buf[0, G:G + 258 * 258] = pad.reshape(-1)
        xbufs.append(buf)

    wts, bts = [], []
    for i in range(6):
        w = np.asarray(inputs[f'conv{i + 1}_w'], dtype=np.float32)
        bias = np.asarray(inputs[f'conv{i + 1}_b'], dtype=np.float32)
        wts.append(np.ascontiguousarray(
            w.transpose(1, 2, 3, 0).reshape(-1, w.shape[0])))
        bts.append(np.ascontiguousarray(bias))

    atlases = []
    for h in range(NUM_HEADS):
        tab = table[:, h].reshape(2 * TABLE_M - 1, 2 * TABLE_M - 1)
        C = tab[96:96 + 127, 96:96 + 127]  # [127, 127]
        tmp = np.zeros((127, 128), np.float32)
        tmp[:, :127] = C
        cfbuf = np.zeros(191 + 16256 + 129, np.float32)
        cfbuf[191:191 + 16256] = tmp.reshape(-1)
        sw = np.lib.stride_tricks.sliding_window_view(cfbuf, 16256)
        p = np.arange(128)
        offs = 254 - (p % 64) - 128 * (p // 64)
        full = sw[offs]                                   # [128, 127*128]
        a2 = full.reshape(128, 127, 128)[:, :, 0:64].reshape(128, 127 * 64)
        atl = np.zeros((128, CF), np.float32)
        atl[:, :127 * 64] = a2
        atlases.append(atl)

    in_maps = []
    for core in range(8):
        b, h = core // 4, core % 4
        m = {"x": xbufs[b], "watlas": atlases[h]}
        for i in range(6):
            m[f"w{i}"] = wts[i]
            m[f"b{i}"] = bts[i]
        m["wq"] = np.ascontiguousarray(qkv_w[h * 64:(h + 1) * 64, :].T)
        m["wk"] = np.ascontiguousarray(qkv_w[256 + h * 64:256 + (h + 1) * 64, :].T)
        m["wv"] = np.ascontiguousarray(qkv_w[512 + h * 64:512 + (h + 1) * 64, :].T)
        in_maps.append(m)
    return in_maps


def kernel(_trace=False, **inputs):
    from concourse.bass_utils import run_bass_kernel_spmd
    nc = _build()
    in_maps = _prep_inputs(inputs)
    import os
    tdir = os.environ.get("KTRACE_DIR")
    if tdir:
        os.makedirs(tdir, exist_ok=True)
    res = run_bass_kernel_spmd(nc, in_maps, core_ids=list(range(8)),
                               trace=_trace, tmpdir=tdir)
    if _trace:
        kernel.last_exec_ns = res.exec_time_ns
        kernel.last_results = res
    # assemble: core -> (b, h): [64(d), 4096(n)]
    O = np.stack([np.stack([res.results[b * 4 + h]["out"] for h in range(4)])
                  for b in range(B)])                      # [B, H, 64, N]
    out = O.transpose(0, 3, 1, 2).reshape(B, N, NUM_HEADS * DIM_HEAD)
    out = out.reshape(B, GRID, GRID, NUM_HEADS * DIM_HEAD)
    shift = int(np.asarray(inputs['window_size'])) // 2
    out = np.roll(out, shift=(-shift, -shift), axis=(1, 2))
    return out.astype(np.float32)
